# revision 5
# baseline (speedup 1.0000x reference)
"""Trainium2 Bass kernel for DifferentiableSupergraphDynamics — v3.

Strategy: destination-sharded (round-robin by in-degree rank) across 8
cores, one canonical padded-CSR layout per core (nodes sorted by total
in-degree, groups of 128 partitions padded to the group max degree).
The host lays x out per edge-slot (a gather is pure data movement), so
the device reads one big contiguous [P, F*B] message-operand stream at
HBM line rate and does all arithmetic: the edge-weight computation
w = where(mask, tanh(theta), sign*conf)*delay, the weighted per-node
segmented reduction, and the clipped nonlinear ODE step.
"""

import numpy as np

import concourse.bass as bass
import concourse.bacc as bacc
import concourse.mybir as mybir
import concourse.tile as tile
from concourse.bass_utils import run_bass_kernel_spmd

P = 128
NCORES = 8
DT = 0.1

F32 = mybir.dt.float32
I8 = mybir.dt.int8


def _equal_d_runs(D):
    runs = []
    a = 0
    G = len(D)
    while a < G:
        b = a + 1
        while b < G and D[b] == D[a]:
            b += 1
        runs.append((a, b, int(D[a])))
        a = b
    return runs


# ---------------------------------------------------------------------------
# Host-side data preparation
# ---------------------------------------------------------------------------

def _prep(x, theta, bias, ratelog, baserate, cap, sign, conf, delay, src, dst,
          mask, n_cores):
    B, N = x.shape
    E = src.shape[0]

    src = np.asarray(src).astype(np.int64)
    dst = np.asarray(dst).astype(np.int64)
    theta = np.asarray(theta, dtype=np.float32)
    sign = np.asarray(sign, dtype=np.float32)
    conf = np.asarray(conf, dtype=np.float32)
    delay = np.asarray(delay, dtype=np.float32)
    mask8 = np.asarray(mask).astype(np.int8)

    deg = np.bincount(dst, minlength=N)
    order = np.argsort(-deg, kind="stable")
    npc = (N + n_cores - 1) // n_cores
    G = (npc + P - 1) // P
    nper = G * P

    rank_of = np.empty(N, dtype=np.int64)
    rank_of[order] = np.arange(N)
    core_of = rank_of % n_cores
    pos_of = rank_of // n_cores              # position-in-core by degree rank

    # canonical layout: position pos -> (partition pos%P, group pos//P)
    # per-core group max degree (shared D over cores so the device program
    # is identical: D[g] = max over cores of that group's max degree)
    deg_core = np.zeros((n_cores, nper), dtype=np.int64)
    np.add.at(deg_core, (core_of[dst], pos_of[dst]), 1)
    D = deg_core.reshape(n_cores, G, P).max(axis=(0, 2))
    D = np.maximum(D, 1)
    S = np.zeros(G + 1, dtype=np.int64)
    S[1:] = np.cumsum(D)
    F = int(S[-1])

    # edge -> slot
    eord = np.lexsort((src, dst))
    ec = core_of[dst][eord]
    ep = pos_of[dst][eord]
    key_change = np.ones(E, dtype=bool)
    key_change[1:] = dst[eord][1:] != dst[eord][:-1]
    run_id = np.cumsum(key_change) - 1
    run_starts = np.flatnonzero(key_change)
    occ = np.arange(E) - run_starts[run_id]

    g = ep // P
    pp = ep % P
    col = S[g] + occ

    par_shape = (n_cores, P, F)
    thetaA = np.zeros(par_shape, np.float32)
    signA = np.zeros(par_shape, np.float32)
    confA = np.zeros(par_shape, np.float32)
    delayA = np.zeros(par_shape, np.float32)
    maskA = np.zeros(par_shape, np.int8)
    pidx = (ec, pp, col)
    thetaA[pidx] = theta[eord]
    signA[pidx] = sign[eord]
    confA[pidx] = conf[eord]
    delayA[pidx] = delay[eord]
    maskA[pidx] = mask8[eord]

    # pre-gathered x per slot: [cores, P, F*B] in batch-major-within-node
    # layout: group g's segment is [S[g]*B, S[g+1]*B); value (b, j) of the
    # node at (p, g) sits at S[g]*B + b*D[g] + j  (j = occurrence).
    xT = np.ascontiguousarray(np.asarray(x, np.float32).T)   # [N, B]
    xslots = np.zeros((n_cores, P, F * B), np.float32)
    De = D[g]
    flat = (S[g] * B + occ)[:, None] + np.arange(B)[None, :] * De[:, None]
    xslots[ec[:, None], pp[:, None], flat] = xT[src[eord], :]

    # node params in canonical placement [P, G]; node at (p, g) has
    # position pos = g*P + p ... careful: pos -> (pp=pos%P, g=pos//P)
    def node_arr(vals, fill):
        a = np.full((n_cores, P, G), fill, dtype=np.float32)
        pos = np.arange(nper)
        for c in range(n_cores):
            rank = pos * n_cores + c
            ok = rank < N
            nd = order[np.minimum(rank, N - 1)]
            v = np.where(ok, vals[nd], fill).astype(np.float32)
            a[c, pos % P, pos // P] = v
        return a

    biasA = node_arr(np.asarray(bias), 0.0)
    ratelogA = node_arr(np.asarray(ratelog), 0.0)
    baserateA = node_arr(np.asarray(baserate), 0.0)
    capA = node_arr(np.asarray(cap), 1.0)

    node_ids = np.zeros((n_cores, P, G), np.int64)
    xownA = np.zeros((n_cores, P, G, B), np.float32)
    pos = np.arange(nper)
    for c in range(n_cores):
        rank = pos * n_cores + c
        ok = rank < N
        nd = np.where(ok, order[np.minimum(rank, N - 1)], -1)
        node_ids[c, pos % P, pos // P] = nd
        xownA[c, pos % P, pos // P, :] = np.where(
            ok[:, None], xT[np.maximum(nd, 0), :], 0.0)

    ins = []
    for c in range(n_cores):
        ins.append({
            "xs": xslots[c],
            "theta": thetaA[c],
            "sgn": signA[c],
            "conf": confA[c],
            "delay": delayA[c],
            "maskf": maskA[c],
            "bias": biasA[c],
            "ratelog": ratelogA[c],
            "baserate": baserateA[c],
            "cap": capA[c],
            "xown": xownA[c].reshape(P, G * B),
        })
    plan = dict(B=B, N=N, G=G, D=D, S=S, F=F, node_ids=node_ids)
    return ins, plan


def _assemble(results, plan):
    B, N, G = plan["B"], plan["N"], plan["G"]
    out = np.empty((B, N), dtype=np.float32)
    for ci, res in enumerate(results):
        o = res["out"].reshape(P, G, B)
        nid = plan["node_ids"][ci]
        ok = nid >= 0
        out[:, nid[ok]] = o[ok].T
    return out


# ---------------------------------------------------------------------------
# Device kernel
# ---------------------------------------------------------------------------

def build(B, G, D, S, F, enable_asserts=False):
    nc = bacc.Bacc("TRN2", target_bir_lowering=False, debug=False,
                   enable_asserts=enable_asserts)

    xsD = nc.dram_tensor("xs", [P, F * B], F32, kind="ExternalInput")
    thD = nc.dram_tensor("theta", [P, F], F32, kind="ExternalInput")
    sgD = nc.dram_tensor("sgn", [P, F], F32, kind="ExternalInput")
    cfD = nc.dram_tensor("conf", [P, F], F32, kind="ExternalInput")
    dlD = nc.dram_tensor("delay", [P, F], F32, kind="ExternalInput")
    mkD = nc.dram_tensor("maskf", [P, F], I8, kind="ExternalInput")
    biD = nc.dram_tensor("bias", [P, G], F32, kind="ExternalInput")
    rlD = nc.dram_tensor("ratelog", [P, G], F32, kind="ExternalInput")
    brD = nc.dram_tensor("baserate", [P, G], F32, kind="ExternalInput")
    cpD = nc.dram_tensor("cap", [P, G], F32, kind="ExternalInput")
    xoD = nc.dram_tensor("xown", [P, G * B], F32, kind="ExternalInput")
    outD = nc.dram_tensor("out", [P, G * B], F32, kind="ExternalOutput")

    Tanh = mybir.ActivationFunctionType.Tanh
    Exp = mybir.ActivationFunctionType.Exp

    # chunking: split equal-D runs into chunks of <= ~256 slot-cols so the
    # xs stream double-buffers; every chunk is a whole number of groups.
    chunks = []   # (c0, c1, ga, gb, d)
    for (ga, gb, d) in _equal_d_runs(D):
        gsz = max(1, 256 // d)                  # ~256 cols per chunk
        a = ga
        while a < gb:
            b = min(gb, a + gsz)
            chunks.append((int(S[a]), int(S[b]), a, b, d))
            a = b

    # w pieces: ranges of whole chunks, ~5 pieces
    npieces = min(5, len(chunks))
    wpieces = []   # (c0, c1)
    per = (len(chunks) + npieces - 1) // npieces
    for i in range(0, len(chunks), per):
        wpieces.append((chunks[i][0], chunks[min(i + per, len(chunks)) - 1][1]))

    # epilogue batches: group ranges ending at chunk boundaries, ~3 batches
    nb = 3
    gcuts = [0]
    for i in range(1, nb):
        target = G * i // nb
        best = min((ch[3] for ch in chunks), key=lambda g: abs(g - target))
        if best > gcuts[-1]:
            gcuts.append(best)
    gcuts.append(G)
    # map: after which chunk index is group-range [.., gcut) complete
    cut_after = {}
    for ci, (c0, c1, ga, gb, d) in enumerate(chunks):
        for gc in gcuts[1:]:
            if gb == gc:
                cut_after[ci] = gc

    with tile.TileContext(nc) as tc:
        with (
            tc.tile_pool(name="persist", bufs=1) as ppool,
            tc.tile_pool(name="stream", bufs=4) as sp,
            tc.tile_pool(name="gstream", bufs=3) as gp,
            tc.tile_pool(name="tmp", bufs=2) as tp,
        ):
            # node params (small; scalar-engine HWDGE ring)
            bi = ppool.tile([P, G], F32, tag="bi")
            rl = ppool.tile([P, G], F32, tag="rl")
            br = ppool.tile([P, G], F32, tag="br")
            cp = ppool.tile([P, G], F32, tag="cp")
            xo = ppool.tile([P, G * B], F32, tag="xo")
            nc.scalar.dma_start(out=bi[:], in_=biD[:, :])
            nc.scalar.dma_start(out=rl[:], in_=rlD[:, :])
            nc.scalar.dma_start(out=br[:], in_=brD[:, :])
            nc.scalar.dma_start(out=cp[:], in_=cpD[:, :])
            nc.scalar.dma_start(out=xo[:], in_=xoD[:, :])
            rate = ppool.tile([P, G], F32, tag="rate")
            nc.scalar.activation(rate[:], rl[:], Exp)
            nc.vector.tensor_mul(rate[:], rate[:], br[:])
            # A = cap*rate*DT ; Bc = 1 - rate*DT   (per node)
            Atl = ppool.tile([P, G], F32, tag="Atl")
            Btl = ppool.tile([P, G], F32, tag="Btl")
            nc.vector.tensor_scalar_mul(rate[:], rate[:], float(DT))
            nc.vector.tensor_mul(Atl[:], cp[:], rate[:])
            nc.vector.tensor_scalar_mul(Btl[:], rate[:], -1.0)
            nc.vector.tensor_scalar_add(Btl[:], Btl[:], 1.0)

            # --- edge weights (f32, [P, F]) in pieces ---
            w = ppool.tile([P, F], F32, tag="w")
            for pi, (c0, c1) in enumerate(wpieces):
                cols = c1 - c0
                th = tp.tile([P, cols], F32, tag="th", name=f"th{pi}")
                sg = tp.tile([P, cols], F32, tag="sg", name=f"sg{pi}")
                cf = tp.tile([P, cols], F32, tag="cf", name=f"cf{pi}")
                dl = tp.tile([P, cols], F32, tag="dl", name=f"dl{pi}")
                mk = tp.tile([P, cols], I8, tag="mk", name=f"mk{pi}")
                nc.scalar.dma_start(out=th[:], in_=thD[:, c0:c1])
                nc.scalar.dma_start(out=sg[:], in_=sgD[:, c0:c1])
                nc.scalar.dma_start(out=cf[:], in_=cfD[:, c0:c1])
                nc.scalar.dma_start(out=dl[:], in_=dlD[:, c0:c1])
                nc.scalar.dma_start(out=mk[:], in_=mkD[:, c0:c1])
                t = tp.tile([P, cols], F32, tag="t", name=f"t{pi}")
                nc.scalar.activation(t[:], th[:], Tanh)
                nc.vector.tensor_mul(w[:, c0:c1], sg[:], cf[:])
                nc.vector.copy_predicated(w[:, c0:c1], mk[:], t[:])
                nc.vector.tensor_mul(w[:, c0:c1], w[:, c0:c1], dl[:])

            agg = ppool.tile([P, G * B], F32, tag="agg")
            dr = ppool.tile([P, G * B], F32, tag="dr")
            ut = ppool.tile([P, G * B], F32, tag="ut")

            def epilogue(g0, g1):
                ncols = (g1 - g0) * B
                a3 = agg[:, g0 * B:g1 * B].rearrange("p (g b) -> p g b", b=B)
                d3 = dr[:, g0 * B:g1 * B].rearrange("p (g b) -> p g b", b=B)
                u3 = ut[:, g0 * B:g1 * B].rearrange("p (g b) -> p g b", b=B)
                drf = dr[:, g0 * B:g1 * B]
                sl = slice(g0, g1)
                bib = bi[:, sl].unsqueeze(-1).to_broadcast([P, g1 - g0, B])
                Ab = Atl[:, sl].unsqueeze(-1).to_broadcast([P, g1 - g0, B])
                Bb = Btl[:, sl].unsqueeze(-1).to_broadcast([P, g1 - g0, B])
                cpb = cp[:, sl].unsqueeze(-1).to_broadcast([P, g1 - g0, B])
                xof = xo[:, g0 * B:g1 * B]
                x3 = xof.rearrange("p (g b) -> p g b", b=B)
                nc.vector.tensor_add(d3, a3, bib)
                nc.scalar.activation(drf, drf, Tanh)
                nc.vector.tensor_mul(d3, d3, Ab)
                nc.vector.tensor_tensor(out=u3, in0=x3, in1=Bb,
                                        op=mybir.AluOpType.mult)
                nc.vector.tensor_add(drf, drf, ut[:, g0 * B:g1 * B])
                nc.vector.tensor_scalar_max(drf, drf, 0.0)
                nc.vector.tensor_tensor(out=d3, in0=d3, in1=cpb,
                                        op=mybir.AluOpType.min)
                nc.sync.dma_start(out=outD[:, g0 * B:g1 * B], in_=drf)

            # --- stream the pre-gathered messages ---
            # every 3rd chunk's multiply runs on gpsimd; its reduce is
            # deferred 2 chunks in the DVE stream so DVE never stalls on
            # the slower gpsimd op. gstream tiles stay live until then.
            gdone = 0
            nred = [0] * len(chunks)    # 1 when chunk's reduce emitted
            pending = []                # (ci, m4, ga, gb)
            epi_cuts = sorted((ci, gc) for ci, gc in cut_after.items())

            def reduce_of(m4, ga, gb):
                dst_ap = agg[:, ga * B:gb * B].rearrange(
                    "p (n b) -> p n b", b=B)
                nc.vector.tensor_reduce(
                    dst_ap, m4, axis=mybir.AxisListType.X,
                    op=mybir.AluOpType.add)

            def fire_epilogues():
                nonlocal gdone
                upto = 0
                while upto < len(chunks) and nred[upto]:
                    upto += 1
                for ci2, gc in epi_cuts:
                    if gc > gdone and ci2 < upto:
                        epilogue(gdone, gc)
                        gdone = gc

            for ci, (c0, c1, ga, gb, d) in enumerate(chunks):
                cols = c1 - c0
                is_g = (ci % 3 == 2)
                pool = gp if is_g else sp
                xs = pool.tile([P, 4096], F32,
                               tag="gxs" if is_g else "xs", name=f"xs{ci}")
                nc.sync.dma_start(out=xs[:, :cols * B],
                                  in_=xsD[:, c0 * B:c1 * B])
                m4 = xs[:, :cols * B].rearrange(
                    "p (n b d) -> p n b d", b=B, d=d)
                wb = (w[:, c0:c1].rearrange("p (n d) -> p n d", d=d)
                      .unsqueeze(2).to_broadcast([P, gb - ga, B, d]))
                if is_g:
                    nc.gpsimd.tensor_tensor(out=m4, in0=m4, in1=wb,
                                            op=mybir.AluOpType.mult)
                    pending.append((ci, m4, ga, gb))
                else:
                    nc.vector.tensor_tensor(out=m4, in0=m4, in1=wb,
                                            op=mybir.AluOpType.mult)
                    reduce_of(m4, ga, gb)
                    nred[ci] = 1
                while pending and pending[0][0] <= ci - 2:
                    pci, pm4, pga, pgb = pending.pop(0)
                    reduce_of(pm4, pga, pgb)
                    nred[pci] = 1
                fire_epilogues()
            for pci, pm4, pga, pgb in pending:
                reduce_of(pm4, pga, pgb)
                nred[pci] = 1
            fire_epilogues()
            if gdone < G:
                epilogue(gdone, G)

    nc.compile()
    return nc


def kernel(x, theta_graph, node_bias, rate_log_scale, base_rate, capacity,
           sign_prior, conf_scale, delay_scale, src_index, dst_index,
           learn_mask):
    ins, plan = _prep(x, theta_graph, node_bias, rate_log_scale, base_rate,
                      capacity, sign_prior, conf_scale, delay_scale,
                      src_index, dst_index, learn_mask, NCORES)
    nc = build(plan["B"], plan["G"], plan["D"], plan["S"], plan["F"])
    res = run_bass_kernel_spmd(nc, ins, core_ids=list(range(NCORES)))
    return _assemble(res.results, plan)


# revision 6
# speedup vs baseline: 1.0649x; 1.0649x over previous
"""Trainium2 Bass kernel for DifferentiableSupergraphDynamics — v3.

Strategy: destination-sharded (round-robin by in-degree rank) across 8
cores, one canonical padded-CSR layout per core (nodes sorted by total
in-degree, groups of 128 partitions padded to the group max degree).
The host lays x out per edge-slot (a gather is pure data movement), so
the device reads one big contiguous [P, F*B] message-operand stream at
HBM line rate and does all arithmetic: the edge-weight computation
w = where(mask, tanh(theta), sign*conf)*delay, the weighted per-node
segmented reduction, and the clipped nonlinear ODE step.
"""

import numpy as np

import concourse.bass as bass
import concourse.bacc as bacc
import concourse.mybir as mybir
import concourse.tile as tile
from concourse.bass_utils import run_bass_kernel_spmd

P = 128
NCORES = 8
DT = 0.1

F32 = mybir.dt.float32
I8 = mybir.dt.int8


def _equal_d_runs(D):
    runs = []
    a = 0
    G = len(D)
    while a < G:
        b = a + 1
        while b < G and D[b] == D[a]:
            b += 1
        runs.append((a, b, int(D[a])))
        a = b
    return runs


# ---------------------------------------------------------------------------
# Host-side data preparation
# ---------------------------------------------------------------------------

def _prep(x, theta, bias, ratelog, baserate, cap, sign, conf, delay, src, dst,
          mask, n_cores):
    B, N = x.shape
    E = src.shape[0]

    src = np.asarray(src).astype(np.int64)
    dst = np.asarray(dst).astype(np.int64)
    theta = np.asarray(theta, dtype=np.float32)
    sign = np.asarray(sign, dtype=np.float32)
    conf = np.asarray(conf, dtype=np.float32)
    delay = np.asarray(delay, dtype=np.float32)
    mask8 = np.asarray(mask).astype(np.int8)

    deg = np.bincount(dst, minlength=N)
    order = np.argsort(-deg, kind="stable")
    npc = (N + n_cores - 1) // n_cores
    G = (npc + P - 1) // P
    nper = G * P

    rank_of = np.empty(N, dtype=np.int64)
    rank_of[order] = np.arange(N)
    core_of = rank_of % n_cores
    pos_of = rank_of // n_cores              # position-in-core by degree rank

    # canonical layout: position pos -> (partition pos%P, group pos//P)
    # per-core group max degree (shared D over cores so the device program
    # is identical: D[g] = max over cores of that group's max degree)
    deg_core = np.zeros((n_cores, nper), dtype=np.int64)
    np.add.at(deg_core, (core_of[dst], pos_of[dst]), 1)
    D = deg_core.reshape(n_cores, G, P).max(axis=(0, 2))
    D = np.maximum(D, 1)
    S = np.zeros(G + 1, dtype=np.int64)
    S[1:] = np.cumsum(D)
    F = int(S[-1])

    # edge -> slot
    eord = np.lexsort((src, dst))
    ec = core_of[dst][eord]
    ep = pos_of[dst][eord]
    key_change = np.ones(E, dtype=bool)
    key_change[1:] = dst[eord][1:] != dst[eord][:-1]
    run_id = np.cumsum(key_change) - 1
    run_starts = np.flatnonzero(key_change)
    occ = np.arange(E) - run_starts[run_id]

    g = ep // P
    pp = ep % P
    col = S[g] + occ

    par_shape = (n_cores, P, F)
    thetaA = np.zeros(par_shape, np.float32)
    signA = np.zeros(par_shape, np.float32)
    confA = np.zeros(par_shape, np.float32)
    delayA = np.zeros(par_shape, np.float32)
    maskA = np.zeros(par_shape, np.int8)
    pidx = (ec, pp, col)
    thetaA[pidx] = theta[eord]
    signA[pidx] = sign[eord]
    confA[pidx] = conf[eord]
    delayA[pidx] = delay[eord]
    maskA[pidx] = mask8[eord]

    # pre-gathered x per slot: [cores, P, F*B] in batch-major-within-node
    # layout: group g's segment is [S[g]*B, S[g+1]*B); value (b, j) of the
    # node at (p, g) sits at S[g]*B + b*D[g] + j  (j = occurrence).
    xT = np.ascontiguousarray(np.asarray(x, np.float32).T)   # [N, B]
    xslots = np.zeros((n_cores, P, F * B), np.float32)
    De = D[g]
    flat = (S[g] * B + occ)[:, None] + np.arange(B)[None, :] * De[:, None]
    xslots[ec[:, None], pp[:, None], flat] = xT[src[eord], :]

    # node params in canonical placement [P, G]; node at (p, g) has
    # position pos = g*P + p ... careful: pos -> (pp=pos%P, g=pos//P)
    def node_arr(vals, fill):
        a = np.full((n_cores, P, G), fill, dtype=np.float32)
        pos = np.arange(nper)
        for c in range(n_cores):
            rank = pos * n_cores + c
            ok = rank < N
            nd = order[np.minimum(rank, N - 1)]
            v = np.where(ok, vals[nd], fill).astype(np.float32)
            a[c, pos % P, pos // P] = v
        return a

    biasA = node_arr(np.asarray(bias), 0.0)
    ratelogA = node_arr(np.asarray(ratelog), 0.0)
    baserateA = node_arr(np.asarray(baserate), 0.0)
    capA = node_arr(np.asarray(cap), 1.0)

    node_ids = np.zeros((n_cores, P, G), np.int64)
    xownA = np.zeros((n_cores, P, G, B), np.float32)
    pos = np.arange(nper)
    for c in range(n_cores):
        rank = pos * n_cores + c
        ok = rank < N
        nd = np.where(ok, order[np.minimum(rank, N - 1)], -1)
        node_ids[c, pos % P, pos // P] = nd
        xownA[c, pos % P, pos // P, :] = np.where(
            ok[:, None], xT[np.maximum(nd, 0), :], 0.0)

    ins = []
    for c in range(n_cores):
        ins.append({
            "xs": xslots[c],
            "theta": thetaA[c],
            "sgn": signA[c],
            "conf": confA[c],
            "delay": delayA[c],
            "maskf": maskA[c],
            "bias": biasA[c],
            "ratelog": ratelogA[c],
            "baserate": baserateA[c],
            "cap": capA[c],
            "xown": xownA[c].reshape(P, G * B),
        })
    plan = dict(B=B, N=N, G=G, D=D, S=S, F=F, node_ids=node_ids)
    return ins, plan


def _assemble(results, plan):
    B, N, G = plan["B"], plan["N"], plan["G"]
    out = np.empty((B, N), dtype=np.float32)
    for ci, res in enumerate(results):
        o = res["out"].reshape(P, G, B)
        nid = plan["node_ids"][ci]
        ok = nid >= 0
        out[:, nid[ok]] = o[ok].T
    return out


# ---------------------------------------------------------------------------
# Device kernel
# ---------------------------------------------------------------------------

def build(B, G, D, S, F, enable_asserts=False):
    nc = bacc.Bacc("TRN2", target_bir_lowering=False, debug=False,
                   enable_asserts=enable_asserts)

    xsD = nc.dram_tensor("xs", [P, F * B], F32, kind="ExternalInput")
    thD = nc.dram_tensor("theta", [P, F], F32, kind="ExternalInput")
    sgD = nc.dram_tensor("sgn", [P, F], F32, kind="ExternalInput")
    cfD = nc.dram_tensor("conf", [P, F], F32, kind="ExternalInput")
    dlD = nc.dram_tensor("delay", [P, F], F32, kind="ExternalInput")
    mkD = nc.dram_tensor("maskf", [P, F], I8, kind="ExternalInput")
    biD = nc.dram_tensor("bias", [P, G], F32, kind="ExternalInput")
    rlD = nc.dram_tensor("ratelog", [P, G], F32, kind="ExternalInput")
    brD = nc.dram_tensor("baserate", [P, G], F32, kind="ExternalInput")
    cpD = nc.dram_tensor("cap", [P, G], F32, kind="ExternalInput")
    xoD = nc.dram_tensor("xown", [P, G * B], F32, kind="ExternalInput")
    outD = nc.dram_tensor("out", [P, G * B], F32, kind="ExternalOutput")

    Tanh = mybir.ActivationFunctionType.Tanh
    Exp = mybir.ActivationFunctionType.Exp

    # chunking: split equal-D runs into chunks of <= ~256 slot-cols so the
    # xs stream double-buffers; every chunk is a whole number of groups.
    chunks = []   # (c0, c1, ga, gb, d)
    for (ga, gb, d) in _equal_d_runs(D):
        gsz = max(1, 512 // d)                  # ~512 cols per chunk
        a = ga
        while a < gb:
            b = min(gb, a + gsz)
            chunks.append((int(S[a]), int(S[b]), a, b, d))
            a = b

    # w pieces: ranges of whole chunks, ~5 pieces
    npieces = min(5, len(chunks))
    wpieces = []   # (c0, c1)
    per = (len(chunks) + npieces - 1) // npieces
    for i in range(0, len(chunks), per):
        wpieces.append((chunks[i][0], chunks[min(i + per, len(chunks)) - 1][1]))

    # epilogue batches: group ranges ending at chunk boundaries, ~3 batches
    nb = 3
    gcuts = [0]
    for i in range(1, nb):
        target = G * i // nb
        best = min((ch[3] for ch in chunks), key=lambda g: abs(g - target))
        if best > gcuts[-1]:
            gcuts.append(best)
    gcuts.append(G)
    # map: after which chunk index is group-range [.., gcut) complete
    cut_after = {}
    for ci, (c0, c1, ga, gb, d) in enumerate(chunks):
        for gc in gcuts[1:]:
            if gb == gc:
                cut_after[ci] = gc

    with tile.TileContext(nc) as tc:
        with (
            tc.tile_pool(name="persist", bufs=1) as ppool,
            tc.tile_pool(name="stream", bufs=3) as sp,
            tc.tile_pool(name="tmp", bufs=2) as tp,
        ):
            # node params (small; scalar-engine HWDGE ring)
            bi = ppool.tile([P, G], F32, tag="bi")
            rl = ppool.tile([P, G], F32, tag="rl")
            br = ppool.tile([P, G], F32, tag="br")
            cp = ppool.tile([P, G], F32, tag="cp")
            xo = ppool.tile([P, G * B], F32, tag="xo")
            nc.scalar.dma_start(out=bi[:], in_=biD[:, :])
            nc.scalar.dma_start(out=rl[:], in_=rlD[:, :])
            nc.scalar.dma_start(out=br[:], in_=brD[:, :])
            nc.scalar.dma_start(out=cp[:], in_=cpD[:, :])
            nc.scalar.dma_start(out=xo[:], in_=xoD[:, :])
            rate = ppool.tile([P, G], F32, tag="rate")
            nc.scalar.activation(rate[:], rl[:], Exp)
            nc.vector.tensor_mul(rate[:], rate[:], br[:])
            # A = cap*rate*DT ; Bc = 1 - rate*DT   (per node)
            Atl = ppool.tile([P, G], F32, tag="Atl")
            Btl = ppool.tile([P, G], F32, tag="Btl")
            nc.vector.tensor_scalar_mul(rate[:], rate[:], float(DT))
            nc.vector.tensor_mul(Atl[:], cp[:], rate[:])
            nc.vector.tensor_scalar_mul(Btl[:], rate[:], -1.0)
            nc.vector.tensor_scalar_add(Btl[:], Btl[:], 1.0)

            # --- edge weights (f32, [P, F]) in pieces ---
            w = ppool.tile([P, F], F32, tag="w")
            for pi, (c0, c1) in enumerate(wpieces):
                cols = c1 - c0
                th = tp.tile([P, cols], F32, tag="th", name=f"th{pi}")
                sg = tp.tile([P, cols], F32, tag="sg", name=f"sg{pi}")
                cf = tp.tile([P, cols], F32, tag="cf", name=f"cf{pi}")
                dl = tp.tile([P, cols], F32, tag="dl", name=f"dl{pi}")
                mk = tp.tile([P, cols], I8, tag="mk", name=f"mk{pi}")
                nc.scalar.dma_start(out=th[:], in_=thD[:, c0:c1])
                nc.scalar.dma_start(out=sg[:], in_=sgD[:, c0:c1])
                nc.scalar.dma_start(out=cf[:], in_=cfD[:, c0:c1])
                nc.scalar.dma_start(out=dl[:], in_=dlD[:, c0:c1])
                nc.scalar.dma_start(out=mk[:], in_=mkD[:, c0:c1])
                t = tp.tile([P, cols], F32, tag="t", name=f"t{pi}")
                nc.scalar.activation(t[:], th[:], Tanh)
                nc.vector.tensor_mul(w[:, c0:c1], sg[:], cf[:])
                nc.vector.copy_predicated(w[:, c0:c1], mk[:], t[:])
                nc.vector.tensor_mul(w[:, c0:c1], w[:, c0:c1], dl[:])

            agg = ppool.tile([P, G * B], F32, tag="agg")
            dr = ppool.tile([P, G * B], F32, tag="dr")
            ut = ppool.tile([P, G * B], F32, tag="ut")

            def epilogue(g0, g1):
                ncols = (g1 - g0) * B
                a3 = agg[:, g0 * B:g1 * B].rearrange("p (g b) -> p g b", b=B)
                d3 = dr[:, g0 * B:g1 * B].rearrange("p (g b) -> p g b", b=B)
                u3 = ut[:, g0 * B:g1 * B].rearrange("p (g b) -> p g b", b=B)
                drf = dr[:, g0 * B:g1 * B]
                sl = slice(g0, g1)
                bib = bi[:, sl].unsqueeze(-1).to_broadcast([P, g1 - g0, B])
                Ab = Atl[:, sl].unsqueeze(-1).to_broadcast([P, g1 - g0, B])
                Bb = Btl[:, sl].unsqueeze(-1).to_broadcast([P, g1 - g0, B])
                cpb = cp[:, sl].unsqueeze(-1).to_broadcast([P, g1 - g0, B])
                xof = xo[:, g0 * B:g1 * B]
                x3 = xof.rearrange("p (g b) -> p g b", b=B)
                nc.vector.tensor_add(d3, a3, bib)
                nc.scalar.activation(drf, drf, Tanh)
                nc.vector.tensor_mul(d3, d3, Ab)
                nc.vector.tensor_tensor(out=u3, in0=x3, in1=Bb,
                                        op=mybir.AluOpType.mult)
                nc.vector.tensor_add(drf, drf, ut[:, g0 * B:g1 * B])
                nc.vector.tensor_scalar_max(drf, drf, 0.0)
                nc.vector.tensor_tensor(out=d3, in0=d3, in1=cpb,
                                        op=mybir.AluOpType.min)
                nc.sync.dma_start(out=outD[:, g0 * B:g1 * B], in_=drf)

            # --- stream the pre-gathered messages ---
            gdone = 0
            for ci, (c0, c1, ga, gb, d) in enumerate(chunks):
                cols = c1 - c0
                xs = sp.tile([P, 8192], F32, tag="xs", name=f"xs{ci}")
                nc.sync.dma_start(out=xs[:, :cols * B],
                                  in_=xsD[:, c0 * B:c1 * B])
                m4 = xs[:, :cols * B].rearrange(
                    "p (n b d) -> p n b d", b=B, d=d)
                wb = (w[:, c0:c1].rearrange("p (n d) -> p n d", d=d)
                      .unsqueeze(2).to_broadcast([P, gb - ga, B, d]))
                nc.vector.tensor_tensor(out=m4, in0=m4, in1=wb,
                                        op=mybir.AluOpType.mult)
                dst_ap = agg[:, ga * B:gb * B].rearrange(
                    "p (n b) -> p n b", b=B)
                nc.vector.tensor_reduce(
                    dst_ap, m4, axis=mybir.AxisListType.X,
                    op=mybir.AluOpType.add)
                gc = cut_after.get(ci)
                if gc is not None:
                    epilogue(gdone, gc)
                    gdone = gc
            if gdone < G:
                epilogue(gdone, G)

    nc.compile()
    return nc


def kernel(x, theta_graph, node_bias, rate_log_scale, base_rate, capacity,
           sign_prior, conf_scale, delay_scale, src_index, dst_index,
           learn_mask):
    ins, plan = _prep(x, theta_graph, node_bias, rate_log_scale, base_rate,
                      capacity, sign_prior, conf_scale, delay_scale,
                      src_index, dst_index, learn_mask, NCORES)
    nc = build(plan["B"], plan["G"], plan["D"], plan["S"], plan["F"])
    res = run_bass_kernel_spmd(nc, ins, core_ids=list(range(NCORES)))
    return _assemble(res.results, plan)
